# revision 1
# baseline (speedup 1.0000x reference)
"""Dense causal MHA (B=2, S=2048, H=16, D=128, hidden=2048) on 8 Trainium2 cores.

Sharding: data-parallel over batch (2) x tensor-parallel over head groups
(4 heads/core).  Core c handles batch c//4, heads 4*(c%4) .. 4*(c%4)+3.
Each core computes a partial output (its heads' contribution to the out
projection, with bo/4 folded in); the host sums the 4 partials per batch.

Kernel layout choices (all matmuls hit the fp32r 1-cycle/row fast path):
  - X^T, Wq^T/Wk^T/Wv^T, Wo^T marshalled on host; Q^T/K^T computed in
    [d, s] layout, V in [s, d] layout.
  - scores are computed transposed (S^T[kv, q]) so the softmax sum runs on
    the PE via a ones-vector matmul; 1/sum is broadcast across partitions
    via a rank-1 ones matmul.
  - 1/sqrt(d) folded into Wq/bq on host; RoPE tables (cos, sign-folded sin)
    precomputed on host in [d, s] layout.
  - causal masking: per 512-column q-chunk only kv tiles below the diagonal
    are computed; diagonal tiles restrict matmul columns and apply one
    shared [128,128] triangular mask in-place on the exp tile.
"""

import sys

sys.path.insert(0, "/opt/trn_rl_repo")

from contextlib import ExitStack

import numpy as np

import concourse.tile as tile
from concourse import bacc, mybir
from concourse.bass_utils import run_bass_kernel_spmd

S = 2048
HID = 2048
D = 128
LH = 4            # heads per core
DL = LH * D       # 512 local inner dims
SC = 512          # q/s chunk
NSC = S // SC     # 4
HCH = HID // 128  # 16 contraction chunks
N_CORES = 8

f32 = mybir.dt.float32
f32r = mybir.dt.float32r
Exp = mybir.ActivationFunctionType.Exp
Ident = mybir.ActivationFunctionType.Identity

_CACHE = {}


def _build_nc():
    nc = bacc.Bacc("TRN2", target_bir_lowering=False, debug=False,
                   num_devices=N_CORES)

    def din(name, shape, dt=f32r):
        return nc.dram_tensor(name, shape, dt, kind="ExternalInput").ap()

    xT = din("xT", [HID, S])
    wqT = din("wqT", [HID, DL])
    wkT = din("wkT", [HID, DL])
    wvT = din("wvT", [HID, DL])
    woT = din("woT", [DL, HID])
    bq2 = din("bq2", [128, LH], f32)
    bk2 = din("bk2", [128, LH], f32)
    cosT = din("cosT", [128, S], f32)
    sinT = din("sinT", [128, S], f32)
    trim = din("trim", [128, 128], f32)
    onec = din("onec", [128, 1])
    oner = din("oner", [1, 128], f32)
    out = nc.dram_tensor("out", [S, HID], f32, kind="ExternalOutput").ap()

    with tile.TileContext(nc) as tc, ExitStack() as ctx:
        P = ctx.enter_context(tc.tile_pool(name="persist", bufs=1))
        cos_sb = P.tile([128, S], f32, tag="cos")
        nc.sync.dma_start(cos_sb[:], cosT[:])
        sin_sb = P.tile([128, S], f32, tag="sin")
        nc.sync.dma_start(sin_sb[:], sinT[:])
        tri_sb = P.tile([128, 128], f32, tag="tri")
        nc.sync.dma_start(tri_sb[:], trim[:])
        bq_sb = P.tile([128, LH], f32, tag="bq")
        nc.sync.dma_start(bq_sb[:], bq2[:])
        bk_sb = P.tile([128, LH], f32, tag="bk")
        nc.sync.dma_start(bk_sb[:], bk2[:])
        onec_sb = P.tile([128, 1], f32r, tag="onec")
        nc.sync.dma_start(onec_sb[:], onec[:])
        oner_sb = P.tile([1, 128], f32, tag="oner")
        nc.sync.dma_start(oner_sb[:], oner[:])
        K_sb = [P.tile([128, S], f32r, tag=f"K{d}", name=f"Ksb{d}") for d in range(LH)]
        V_sb = [P.tile([128, DL], f32r, tag=f"V{t}", name=f"Vsb{t}") for t in range(S // 128)]

        def rope(pool, raw, dst, sl):
            """dst = raw*cos + rotate_half(raw)*sin; the rotate is folded
            into partition-offset reads against a half-sign-flipped sin
            table (sin_sb rows >=64 carry the minus sign)."""
            m1 = pool.tile([128, SC], f32, tag="rm1", bufs=2)
            nc.vector.tensor_mul(m1[:], raw[:], cos_sb[:, sl])
            m2 = pool.tile([128, SC], f32, tag="rm2", bufs=2)
            nc.vector.tensor_mul(m2[0:64, :], raw[64:128, :],
                                 sin_sb[64:128, sl])
            nc.vector.tensor_mul(m2[64:128, :], raw[0:64, :],
                                 sin_sb[0:64, sl])
            nc.vector.tensor_add(dst, m1[:], m2[:])

        WQ = ctx.enter_context(tc.tile_pool(name="p2wq", bufs=1))
        wq_sb = [WQ.tile([128, DL], f32r, tag=f"wq{h}", name=f"wqsb{h}")
                 for h in range(HCH)]

        # ---- phase 1: K^T (roped) and V for the whole sequence ----
        with tc.tile_pool(name="p1w", bufs=1) as WP, \
             tc.tile_pool(name="p1x", bufs=4) as XP, \
             tc.tile_pool(name="p1t", bufs=2) as TP, \
             tc.tile_pool(name="p1ps", bufs=4, space="PSUM") as PK, \
             tc.tile_pool(name="p1pv", bufs=4, space="PSUM") as PV:
            wk_sb = [WP.tile([128, DL], f32r, tag=f"wk{h}", name=f"wksb{h}") for h in range(HCH)]
            wv_sb = [WP.tile([128, DL], f32r, tag=f"wv{h}", name=f"wvsb{h}") for h in range(HCH)]
            for h in range(HCH):
                nc.sync.dma_start(wk_sb[h][:], wkT[128 * h:128 * (h + 1), :])
                nc.sync.dma_start(wv_sb[h][:], wvT[128 * h:128 * (h + 1), :])
            for j in range(NSC):
                sl = slice(SC * j, SC * (j + 1))
                psk = [PK.tile([128, SC], f32, tag="pk", name=f"psk{j}_{d}") for d in range(LH)]
                psv = [PV.tile([128, DL], f32, tag="pv", name=f"psv{j}_{st}") for st in range(4)]
                for h in range(HCH):
                    xt = XP.tile([128, SC], f32r, tag="xt")
                    nc.sync.dma_start(xt[:], xT[128 * h:128 * (h + 1), sl])
                    for d in range(LH):
                        nc.tensor.matmul(psk[d][:],
                                         wk_sb[h][:, 128 * d:128 * (d + 1)],
                                         xt[:], start=(h == 0),
                                         stop=(h == HCH - 1))
                    for st in range(4):
                        nc.tensor.matmul(psv[st][:],
                                         xt[:, 128 * st:128 * (st + 1)],
                                         wv_sb[h][:], start=(h == 0),
                                         stop=(h == HCH - 1))
                for st in range(4):
                    nc.scalar.copy(V_sb[4 * j + st][:], psv[st][:])
                for d in range(LH):
                    kraw = TP.tile([128, SC], f32, tag="kraw", bufs=2)
                    nc.scalar.activation(kraw[:], psk[d][:], Ident,
                                         bias=bk_sb[:, d:d + 1], scale=1.0)
                    rope(TP, kraw, K_sb[d][:, sl], sl)
                if j == 0:
                    # prefetch Wq during the rest of phase 1
                    for h in range(HCH):
                        nc.sync.dma_start(wq_sb[h][:],
                                          wqT[128 * h:128 * (h + 1), :])

        # ---- phase 2: per q-chunk: Q proj + rope, attention, out proj ----
        with tc.tile_pool(name="p2x", bufs=4) as X2, \
             tc.tile_pool(name="p2t", bufs=2) as T2, \
             tc.tile_pool(name="p2q", bufs=2) as QP, \
             tc.tile_pool(name="p2ex", bufs=4) as EX, \
             tc.tile_pool(name="p2rc", bufs=2) as RC, \
             tc.tile_pool(name="p2dn", bufs=2) as DN, \
             tc.tile_pool(name="p2cu", bufs=2) as CU, \
             tc.tile_pool(name="p2rb", bufs=2) as RB, \
             tc.tile_pool(name="p2ct", bufs=2) as CT, \
             tc.tile_pool(name="p2wo", bufs=4) as WO, \
             tc.tile_pool(name="p2ot", bufs=4) as OT, \
             tc.tile_pool(name="p2pa", bufs=4, space="PSUM") as PA, \
             tc.tile_pool(name="p2ps", bufs=2, space="PSUM") as PS, \
             tc.tile_pool(name="p2pc", bufs=1, space="PSUM") as PC, \
             tc.tile_pool(name="p2pd", bufs=1, space="PSUM") as PD:
            def emit_outproj(j, ct):
                for oc in range(4):
                    osl = slice(SC * oc, SC * (oc + 1))
                    wo_t = [WO.tile([128, SC], f32r, tag="wo",
                                    name=f"wot{j}_{oc}_{it}")
                            for it in range(LH)]
                    for it in range(LH):
                        nc.sync.dma_start(wo_t[it][:],
                                          woT[128 * it:128 * (it + 1), osl])
                    for qt in range(4):
                        pso = PS.tile([128, SC], f32, tag="ps",
                                      name=f"pso{j}_{oc}_{qt}")
                        for it in range(LH):
                            nc.tensor.matmul(
                                pso[:], ct[it][:, 128 * qt:128 * (qt + 1)],
                                wo_t[it][:], start=(it == 0),
                                stop=(it == LH - 1))
                        ot = OT.tile([128, SC], f32, tag="ot",
                                     name=f"ot{j}_{oc}_{qt}")
                        if qt % 2 == 0:
                            nc.scalar.copy(ot[:], pso[:])
                        else:
                            nc.vector.tensor_copy(ot[:], pso[:])
                        nc.sync.dma_start(
                            out[SC * j + 128 * qt:SC * j + 128 * (qt + 1),
                                osl], ot[:])

            pending = None  # (j, ct) outproj deferred one chunk
            for j in range(NSC):
                sl = slice(SC * j, SC * (j + 1))
                # Q projection + rope
                q_sb = [None] * LH
                psq = [PA.tile([128, SC], f32, tag="pA",
                               name=f"psq{j}_{d}") for d in range(LH)]
                for h in range(HCH):
                    xt = X2.tile([128, SC], f32r, tag="xt2",
                                 name=f"xt2_{j}_{h}")
                    nc.sync.dma_start(xt[:], xT[128 * h:128 * (h + 1), sl])
                    for d in range(LH):
                        nc.tensor.matmul(psq[d][:],
                                         wq_sb[h][:, 128 * d:128 * (d + 1)],
                                         xt[:], start=(h == 0),
                                         stop=(h == HCH - 1))
                for d in range(LH):
                    qraw = T2.tile([128, SC], f32, tag="qraw", bufs=2)
                    nc.scalar.activation(qraw[:], psq[d][:], Ident,
                                         bias=bq_sb[:, d:d + 1], scale=1.0)
                    qro = QP.tile([128, SC], f32r, tag=f"q{d}",
                                  name=f"qro{j}_{d}")
                    rope(T2, qraw, qro[:], sl)
                    q_sb[d] = qro
                if pending is not None:
                    emit_outproj(*pending)
                # attention per local head; normalization deferred one
                # head so the PE never waits on the reciprocal chain
                ct = [None] * LH
                T = 4 * j + 4

                def emit_norm(h, ctu, den):
                    rec32 = RC.tile([1, SC], f32, tag="rec",
                                    name=f"rec{j}_{h}")
                    nc.vector.reciprocal_approx_fast(out=rec32[:],
                                                     in_=den[:])
                    psb = PS.tile([128, SC], f32, tag="ps",
                                  name=f"psb{j}_{h}")
                    nc.tensor.matmul(psb[:], oner_sb[:], rec32[:],
                                     start=True, stop=True)
                    rb = RB.tile([128, SC], f32, tag="rb",
                                 name=f"rb{j}_{h}")
                    nc.scalar.copy(rb[:], psb[:])
                    cth = CT.tile([128, SC], f32r, tag=f"ct{h}",
                                  name=f"ct{j}_{h}")
                    nc.vector.tensor_mul(cth[:], ctu[:], rb[:])
                    ct[h] = cth

                pend = []
                for h in range(LH):
                    psc = PC.tile([128, SC], f32, tag="pc", name=f"psc{j}_{h}")
                    psd = PD.tile([1, SC], f32, tag="pd", name=f"psd{j}_{h}")
                    for t in range(T):
                        p = t - 4 * j  # >=0 for diagonal tiles
                        c0 = 128 * p if p > 0 else 0
                        cs = slice(c0, SC)
                        pss = PS.tile([128, SC], f32, tag="ps")
                        nc.tensor.matmul(pss[:, cs],
                                         K_sb[h][:, 128 * t:128 * (t + 1)],
                                         q_sb[h][:, cs], start=True, stop=True)
                        ex = EX.tile([128, SC], f32r, tag="ex")
                        nc.scalar.activation(ex[:, cs], pss[:, cs], Exp)
                        if p >= 0:
                            dsl = slice(128 * p, 128 * (p + 1))
                            nc.vector.tensor_mul(ex[:, dsl], ex[:, dsl],
                                                 tri_sb[:])
                        nc.tensor.matmul(psd[:, cs], onec_sb[:], ex[:, cs],
                                         start=(t == 0), stop=(t == T - 1))
                        nc.tensor.matmul(psc[:, cs],
                                         V_sb[t][:, 128 * h:128 * (h + 1)],
                                         ex[:, cs], start=(t == 0),
                                         stop=(t == T - 1))
                    den = DN.tile([1, SC], f32, tag="den",
                                  name=f"den{j}_{h}")
                    nc.scalar.copy(den[:], psd[:])
                    ctu = CU.tile([128, SC], f32, tag="ctu",
                                  name=f"ctu{j}_{h}")
                    nc.scalar.copy(ctu[:], psc[:])
                    pend.append((h, ctu, den))
                    if len(pend) > 1:
                        emit_norm(*pend.pop(0))
                for args in pend:
                    emit_norm(*args)
                pending = (j, list(ct))
            emit_outproj(*pending)
    nc.compile()
    return nc


def _get_nc():
    if "nc" not in _CACHE:
        _CACHE["nc"] = _build_nc()
    return _CACHE["nc"]


def _consts():
    if "consts" not in _CACHE:
        inv = (10000.0 ** (-np.arange(0, D, 2, dtype=np.float64) / D))
        t = np.arange(S, dtype=np.float64)
        fr = np.outer(t, inv)                      # [S, 64]
        cos = np.concatenate([np.cos(fr)] * 2, 1).T.astype(np.float32)
        sin = np.concatenate([np.sin(fr)] * 2, 1).T.astype(np.float32)
        sin[64:] *= -1.0
        tri = (np.arange(128)[:, None] <= np.arange(128)[None, :])
        _CACHE["consts"] = {
            "cosT": np.ascontiguousarray(cos),
            "sinT": np.ascontiguousarray(sin),
            "trim": np.ascontiguousarray(tri.astype(np.float32)),
            "onec": np.ones((128, 1), np.float32),
            "oner": np.ones((1, 128), np.float32),
        }
    return _CACHE["consts"]


def _marshal(hidden_states, Wq, bq, Wk, bk, Wv, bv, Wo, bo):
    consts = _consts()
    scale = 1.0 / np.sqrt(D)
    xTs = [np.ascontiguousarray(hidden_states[b].T.astype(np.float32))
           for b in range(2)]
    in_maps = []
    for c in range(N_CORES):
        b, hg = c // 4, c % 4
        rows = slice(DL * hg, DL * (hg + 1))
        m = dict(consts)
        m["xT"] = xTs[b]
        m["wqT"] = np.ascontiguousarray((Wq[rows] * scale).T.astype(np.float32))
        m["wkT"] = np.ascontiguousarray(Wk[rows].T.astype(np.float32))
        m["wvT"] = np.ascontiguousarray(Wv[rows].T.astype(np.float32))
        m["woT"] = np.ascontiguousarray(Wo[:, rows].T.astype(np.float32))
        m["bq2"] = np.ascontiguousarray(
            (bq[rows] * scale).reshape(LH, 128).T.astype(np.float32))
        m["bk2"] = np.ascontiguousarray(
            bk[rows].reshape(LH, 128).T.astype(np.float32))
        in_maps.append(m)
    return in_maps


def _gather(results, bias):
    out = np.empty((2, S, HID), np.float32)
    for b in range(2):
        acc = results[4 * b]["out"].astype(np.float32).copy()
        for g in range(1, 4):
            acc += results[4 * b + g]["out"]
        out[b] = acc + bias
    return out


def _run(inputs, **kw):
    nc = _get_nc()
    in_maps = _marshal(**{k: np.asarray(v) for k, v in inputs.items()})
    return run_bass_kernel_spmd(nc, in_maps, core_ids=list(range(N_CORES)),
                                **kw)


def _host_bias(inputs):
    Wo = np.asarray(inputs["Wo"], np.float64)
    bv = np.asarray(inputs["bv"], np.float64)
    bo = np.asarray(inputs["bo"], np.float64)
    return (bo + Wo @ bv).astype(np.float32)


def kernel(**inputs):
    res = _run(inputs)
    return _gather(res.results, _host_bias(inputs))


def kernel_traced(**inputs):
    """Like kernel() but with NTFF profiling; returns (output, results)."""
    import types

    try:
        import antenv.axon_hooks  # noqa: F401
    except ImportError:
        from trn_agent_boot.trn_boot import _ntff_profile_via_ctypes
        hook = _ntff_profile_via_ctypes("/opt/axon/libaxon_pjrt.so")
        mod = types.ModuleType("antenv.axon_hooks")
        mod.get_axon_ntff_profile_hook = lambda: hook
        mod.set_axon_ntff_profile_hook = lambda h: None
        sys.modules["antenv.axon_hooks"] = mod
    res = _run(inputs, trace=True)
    return _gather(res.results, _host_bias(inputs)), res



# revision 2
# speedup vs baseline: 1.5151x; 1.5151x over previous
"""Dense causal MHA (B=2, S=2048, H=16, D=128, hidden=2048) on 8 Trainium2 cores.

Sharding: data-parallel over batch (2) x tensor-parallel over head groups
(4 heads/core).  Core c handles batch c//4, heads 4*(c%4) .. 4*(c%4)+3.
Each core computes a partial output (its heads' contribution to the out
projection, with bo/4 folded in); the host sums the 4 partials per batch.

v2 layout (everything bf16 except PSUM accumulation, denominators and the
final output, which stay f32):
  - single pass over x: phase 1 computes K^T (roped), V and Q^T (roped) for
    the whole sequence, time-multiplexing PSUM banks K->Q within a chunk.
  - phase 2 is attention + out-projection only, software-pipelined with a
    2-tile lookahead so the PE never waits on the exp chain.
  - psum banks phase 2: scores 3 + context 2 + denominator 1 + outproj 2.
  - softmax denominator via ones-vector matmul; 1/sum broadcast across
    partitions on the (otherwise idle) gpsimd engine.
  - out-projection deferred one chunk so its matmuls fill the PE while the
    next chunk's attention warms up.
"""

import sys

sys.path.insert(0, "/opt/trn_rl_repo")

from contextlib import ExitStack

import numpy as np
import ml_dtypes

import concourse.tile as tile
from concourse import bacc, mybir
from concourse.bass_utils import run_bass_kernel_spmd

S = 2048
HID = 2048
D = 128
LH = 4            # heads per core
DL = LH * D       # 512 local inner dims
SC = 512          # chunk size (q and kv)
NSC = S // SC     # 4
HCH = HID // 128  # 16 contraction chunks
N_CORES = 8

f32 = mybir.dt.float32
bf16 = mybir.dt.bfloat16
Exp = mybir.ActivationFunctionType.Exp
Ident = mybir.ActivationFunctionType.Identity

_CACHE = {}


def _build_nc():
    nc = bacc.Bacc("TRN2", target_bir_lowering=False, debug=False,
                   num_devices=N_CORES)

    def din(name, shape, dt=bf16):
        return nc.dram_tensor(name, shape, dt, kind="ExternalInput").ap()

    xT = din("xT", [HID, S])
    wqT = din("wqT", [HID, DL])
    wkT = din("wkT", [HID, DL])
    wvT = din("wvT", [HID, DL])
    woT = din("woT", [DL, HID])
    bq2 = din("bq2", [128, LH], f32)
    bk2 = din("bk2", [128, LH], f32)
    cosT = din("cosT", [128, S])
    sinT = din("sinT", [128, S])
    trim = din("trim", [128, 128])
    onec = din("onec", [128, 1])
    out = nc.dram_tensor("out", [S, HID], f32, kind="ExternalOutput").ap()

    with tile.TileContext(nc) as tc, ExitStack() as ctx:
        P = ctx.enter_context(tc.tile_pool(name="persist", bufs=1))
        WQP = ctx.enter_context(tc.tile_pool(name="wq", bufs=1))
        WOP = ctx.enter_context(tc.tile_pool(name="wo", bufs=1))

        K_sb = [P.tile([128, S], bf16, tag=f"K{d}", name=f"Ksb{d}")
                for d in range(LH)]
        Q_sb = [P.tile([128, S], bf16, tag=f"Q{d}", name=f"Qsb{d}")
                for d in range(LH)]
        V_sb = [P.tile([128, DL], bf16, tag=f"V{t}", name=f"Vsb{t}")
                for t in range(S // 128)]
        cos_sb = P.tile([128, S], bf16, tag="cos")
        sin_sb = P.tile([128, S], bf16, tag="sin")
        tri_sb = P.tile([128, 128], bf16, tag="tri")
        bq_sb = P.tile([128, LH], f32, tag="bq")
        bk_sb = P.tile([128, LH], f32, tag="bk")
        onec_sb = P.tile([128, 1], bf16, tag="onec")
        wo_sb = [WOP.tile([128, HID], bf16, tag=f"wo{h}", name=f"wosb{h}")
                 for h in range(LH)]

        def rope(pool, raw, dst, sl):
            """dst = raw*cos + rotate_half(raw)*sin; the rotate is folded
            into partition-offset reads against a half-sign-flipped sin
            table (sin_sb rows >=64 carry the minus sign)."""
            m1 = pool.tile([128, SC], bf16, tag="rm1", bufs=2)
            nc.vector.tensor_mul(m1[:], raw[:], cos_sb[:, sl])
            m2 = pool.tile([128, SC], bf16, tag="rm2", bufs=2)
            nc.vector.tensor_mul(m2[0:64, :], raw[64:128, :],
                                 sin_sb[64:128, sl])
            nc.vector.tensor_mul(m2[64:128, :], raw[0:64, :],
                                 sin_sb[0:64, sl])
            nc.vector.tensor_add(dst, m1[:], m2[:])

        # ---- phase 1: K^T (roped), V, Q^T (roped) for the whole sequence ----
        with tc.tile_pool(name="p1w", bufs=1) as WP, \
             tc.tile_pool(name="p1x", bufs=1) as XP, \
             tc.tile_pool(name="p1t", bufs=2) as TP, \
             tc.tile_pool(name="p1kq", bufs=4, space="PSUM") as PKQ, \
             tc.tile_pool(name="p1v", bufs=4, space="PSUM") as PV:
            wk_sb = [WP.tile([128, DL], bf16, tag=f"wk{h}", name=f"wksb{h}")
                     for h in range(HCH)]
            wv_sb = [WP.tile([128, DL], bf16, tag=f"wv{h}", name=f"wvsb{h}")
                     for h in range(HCH)]
            wq_sb = [WQP.tile([128, DL], bf16, tag=f"wq{h}", name=f"wqsb{h}")
                     for h in range(HCH)]
            # one chunk of x tiles in flight plus the next being fetched
            x_t = [[XP.tile([128, SC], bf16, tag=f"x{h}_{j % 2}",
                            name=f"xt{j}_{h}") for h in range(HCH)]
                   for j in range(NSC)]

            # interleave the first chunk's x tiles with wk so compute can
            # start ~1us in; everything else follows in need order.
            for h in range(HCH):
                nc.sync.dma_start(wk_sb[h][:], wkT[128 * h:128 * (h + 1), :])
                nc.sync.dma_start(x_t[0][h][:], xT[128 * h:128 * (h + 1),
                                                   0:SC])
            for h in range(HCH):
                nc.sync.dma_start(wv_sb[h][:], wvT[128 * h:128 * (h + 1), :])
            nc.sync.dma_start(cos_sb[:], cosT[:])
            nc.sync.dma_start(sin_sb[:], sinT[:])
            nc.sync.dma_start(tri_sb[:], trim[:])
            nc.sync.dma_start(onec_sb[:], onec[:])
            nc.sync.dma_start(bq_sb[:], bq2[:])
            nc.sync.dma_start(bk_sb[:], bk2[:])
            for h in range(HCH):
                nc.sync.dma_start(wq_sb[h][:], wqT[128 * h:128 * (h + 1), :])
            for h in range(HCH):
                nc.sync.dma_start(x_t[1][h][:], xT[128 * h:128 * (h + 1),
                                                   SC:2 * SC])
            for h in range(LH):
                nc.sync.dma_start(wo_sb[h][:], woT[128 * h:128 * (h + 1), :])

            for j in range(NSC):
                sl = slice(SC * j, SC * (j + 1))
                if j >= 1:
                    nj = j + 1
                    if nj < NSC:
                        for h in range(HCH):
                            nc.sync.dma_start(
                                x_t[nj][h][:],
                                xT[128 * h:128 * (h + 1),
                                   SC * nj:SC * (nj + 1)])
                # K projection (4 banks), then V (4 banks), then Q reusing
                # K's banks after the Act engine drained them.
                psk = [PKQ.tile([128, SC], f32, tag="pkq",
                                name=f"psk{j}_{d}") for d in range(LH)]
                for h in range(HCH):
                    for d in range(LH):
                        nc.tensor.matmul(psk[d][:],
                                         wk_sb[h][:, 128 * d:128 * (d + 1)],
                                         x_t[j][h][:], start=(h == 0),
                                         stop=(h == HCH - 1))
                psv = [PV.tile([128, DL], f32, tag="pv",
                               name=f"psv{j}_{st}") for st in range(4)]
                for h in range(HCH):
                    for st in range(4):
                        nc.tensor.matmul(psv[st][:],
                                         x_t[j][h][:, 128 * st:128 * (st + 1)],
                                         wv_sb[h][:], start=(h == 0),
                                         stop=(h == HCH - 1))
                for d in range(LH):
                    kraw = TP.tile([128, SC], bf16, tag="kraw", bufs=2)
                    nc.scalar.activation(kraw[:], psk[d][:], Ident,
                                         bias=bk_sb[:, d:d + 1], scale=1.0)
                    rope(TP, kraw, K_sb[d][:, sl], sl)
                psq = [PKQ.tile([128, SC], f32, tag="pkq",
                                name=f"psq{j}_{d}") for d in range(LH)]
                for h in range(HCH):
                    for d in range(LH):
                        nc.tensor.matmul(psq[d][:],
                                         wq_sb[h][:, 128 * d:128 * (d + 1)],
                                         x_t[j][h][:], start=(h == 0),
                                         stop=(h == HCH - 1))
                for st in range(4):
                    nc.scalar.copy(V_sb[4 * j + st][:], psv[st][:])
                for d in range(LH):
                    qraw = TP.tile([128, SC], bf16, tag="qraw", bufs=2)
                    nc.scalar.activation(qraw[:], psq[d][:], Ident,
                                         bias=bq_sb[:, d:d + 1], scale=1.0)
                    rope(TP, qraw, Q_sb[d][:, sl], sl)

        # ---- phase 2: attention (pipelined) + deferred out-projection ----
        with tc.tile_pool(name="p2ex", bufs=3) as EX, \
             tc.tile_pool(name="p2dn", bufs=2) as DN, \
             tc.tile_pool(name="p2rc", bufs=2) as RC, \
             tc.tile_pool(name="p2rb", bufs=2) as RB, \
             tc.tile_pool(name="p2ct", bufs=2) as CT, \
             tc.tile_pool(name="p2ot", bufs=4) as OT, \
             tc.tile_pool(name="p2ps", bufs=3, space="PSUM") as PS, \
             tc.tile_pool(name="p2pc", bufs=2, space="PSUM") as PC, \
             tc.tile_pool(name="p2pd", bufs=1, space="PSUM") as PD, \
             tc.tile_pool(name="p2po", bufs=2, space="PSUM") as PO:

            def emit_outproj(j, ct):
                for oc in range(4):
                    osl = slice(SC * oc, SC * (oc + 1))
                    for qt in range(4):
                        pso = PO.tile([128, SC], f32, tag="po",
                                      name=f"pso{j}_{oc}_{qt}")
                        for it in range(LH):
                            nc.tensor.matmul(
                                pso[:], ct[it][:, 128 * qt:128 * (qt + 1)],
                                wo_sb[it][:, osl], start=(it == 0),
                                stop=(it == LH - 1))
                        ot = OT.tile([128, SC], f32, tag="ot",
                                     name=f"ot{j}_{oc}_{qt}")
                        nc.vector.tensor_copy(ot[:], pso[:])
                        nc.sync.dma_start(
                            out[SC * j + 128 * qt:SC * j + 128 * (qt + 1),
                                osl], ot[:])

            pending = None  # (j, ct) outproj deferred one chunk
            for j in range(NSC):
                qsl = slice(SC * j, SC * (j + 1))
                ct = [None] * LH
                T = 4 * j + 4

                for h in range(LH):
                    psc = PC.tile([128, SC], f32, tag="pc",
                                  name=f"psc{j}_{h}")
                    psd = PD.tile([1, SC], f32, tag="pd", name=f"psd{j}_{h}")
                    pss = [None] * T
                    exs = [None] * T

                    def emit_score(t):
                        p = t - 4 * j  # >=0 for diagonal tiles
                        c0 = 128 * p if p > 0 else 0
                        cs = slice(c0, SC)
                        ps = PS.tile([128, SC], f32, tag="ps",
                                     name=f"pss{j}_{h}_{t}")
                        nc.tensor.matmul(ps[:, cs],
                                         K_sb[h][:, 128 * t:128 * (t + 1)],
                                         Q_sb[h][:, SC * j + c0:SC * (j + 1)],
                                         start=True, stop=True)
                        ex = EX.tile([128, SC], bf16, tag="ex",
                                     name=f"ex{j}_{h}_{t}")
                        nc.scalar.activation(ex[:, cs], ps[:, cs], Exp)
                        if p >= 0:
                            dsl = slice(128 * p, 128 * (p + 1))
                            nc.vector.tensor_mul(ex[:, dsl], ex[:, dsl],
                                                 tri_sb[:])
                        pss[t], exs[t] = ps, ex

                    emit_score(0)
                    if T > 1:
                        emit_score(1)
                    for t in range(T):
                        if t + 2 < T:
                            emit_score(t + 2)
                        p = t - 4 * j
                        cs = slice(128 * p if p > 0 else 0, SC)
                        nc.tensor.matmul(psd[:, cs], onec_sb[:],
                                         exs[t][:, cs],
                                         start=(t == 0), stop=(t == T - 1))
                        nc.tensor.matmul(psc[:, cs],
                                         V_sb[t][:, 128 * h:128 * (h + 1)],
                                         exs[t][:, cs], start=(t == 0),
                                         stop=(t == T - 1))
                    den = DN.tile([1, SC], f32, tag="den",
                                  name=f"den{j}_{h}")
                    nc.vector.tensor_copy(den[:], psd[:])
                    rec = RC.tile([1, SC], f32, tag="rec",
                                  name=f"rec{j}_{h}")
                    nc.vector.reciprocal_approx_fast(out=rec[:], in_=den[:])
                    rb = RB.tile([128, SC], f32, tag="rb",
                                 name=f"rb{j}_{h}")
                    nc.gpsimd.partition_broadcast(rb[:], rec[:])
                    cth = CT.tile([128, SC], bf16, tag=f"ct{h}",
                                  name=f"ct{j}_{h}")
                    nc.vector.tensor_mul(cth[:], psc[:], rb[:])
                    ct[h] = cth
                if pending is not None:
                    emit_outproj(*pending)
                pending = (j, list(ct))
            emit_outproj(*pending)
    nc.compile()
    return nc


def _get_nc():
    if "nc" not in _CACHE:
        _CACHE["nc"] = _build_nc()
    return _CACHE["nc"]


def _consts():
    if "consts" not in _CACHE:
        inv = (10000.0 ** (-np.arange(0, D, 2, dtype=np.float64) / D))
        t = np.arange(S, dtype=np.float64)
        fr = np.outer(t, inv)                      # [S, 64]
        cos = np.concatenate([np.cos(fr)] * 2, 1).T
        sin = np.concatenate([np.sin(fr)] * 2, 1).T.copy()
        sin[64:] *= -1.0
        tri = (np.arange(128)[:, None] <= np.arange(128)[None, :])
        _CACHE["consts"] = {
            "cosT": np.ascontiguousarray(cos.astype(ml_dtypes.bfloat16)),
            "sinT": np.ascontiguousarray(sin.astype(ml_dtypes.bfloat16)),
            "trim": np.ascontiguousarray(
                tri.astype(ml_dtypes.bfloat16)),
            "onec": np.ones((128, 1), ml_dtypes.bfloat16),
        }
    return _CACHE["consts"]


def _marshal(hidden_states, Wq, bq, Wk, bk, Wv, bv, Wo, bo):
    consts = _consts()
    scale = 1.0 / np.sqrt(D)
    xTs = [np.ascontiguousarray(
        hidden_states[b].T.astype(ml_dtypes.bfloat16)) for b in range(2)]
    in_maps = []
    for c in range(N_CORES):
        b, hg = c // 4, c % 4
        rows = slice(DL * hg, DL * (hg + 1))
        m = dict(consts)
        m["xT"] = xTs[b]
        m["wqT"] = np.ascontiguousarray(
            (Wq[rows] * scale).T.astype(ml_dtypes.bfloat16))
        m["wkT"] = np.ascontiguousarray(Wk[rows].T.astype(ml_dtypes.bfloat16))
        m["wvT"] = np.ascontiguousarray(Wv[rows].T.astype(ml_dtypes.bfloat16))
        m["woT"] = np.ascontiguousarray(
            Wo[:, rows].T.astype(ml_dtypes.bfloat16))
        m["bq2"] = np.ascontiguousarray(
            (bq[rows] * scale).reshape(LH, 128).T.astype(np.float32))
        m["bk2"] = np.ascontiguousarray(
            bk[rows].reshape(LH, 128).T.astype(np.float32))
        in_maps.append(m)
    return in_maps


def _gather(results, bias):
    out = np.empty((2, S, HID), np.float32)
    for b in range(2):
        acc = results[4 * b]["out"].astype(np.float32).copy()
        for g in range(1, 4):
            acc += results[4 * b + g]["out"]
        out[b] = acc + bias
    return out


def _run(inputs, **kw):
    nc = _get_nc()
    in_maps = _marshal(**{k: np.asarray(v) for k, v in inputs.items()})
    return run_bass_kernel_spmd(nc, in_maps, core_ids=list(range(N_CORES)),
                                **kw)


def _host_bias(inputs):
    Wo = np.asarray(inputs["Wo"], np.float64)
    bv = np.asarray(inputs["bv"], np.float64)
    bo = np.asarray(inputs["bo"], np.float64)
    return (bo + Wo @ bv).astype(np.float32)


def kernel(**inputs):
    res = _run(inputs)
    return _gather(res.results, _host_bias(inputs))


def kernel_traced(**inputs):
    """Like kernel() but with NTFF profiling; returns (output, results)."""
    import types

    try:
        import antenv.axon_hooks  # noqa: F401
    except ImportError:
        from trn_agent_boot.trn_boot import _ntff_profile_via_ctypes
        hook = _ntff_profile_via_ctypes("/opt/axon/libaxon_pjrt.so")
        mod = types.ModuleType("antenv.axon_hooks")
        mod.get_axon_ntff_profile_hook = lambda: hook
        mod.set_axon_ntff_profile_hook = lambda h: None
        sys.modules["antenv.axon_hooks"] = mod
    res = _run(inputs, trace=True)
    return _gather(res.results, _host_bias(inputs)), res
